# revision 15
# baseline (speedup 1.0000x reference)
"""Self-contained Trainium2 Bass kernel: pre-LN multi-head attention block.

Computes, for x [B=8, S=1024, D=1024] (fp32) and packed attention weights:
    out = x + out_proj(MHA(LayerNorm(x)))
matching torch nn.MultiheadAttention's explicit (non-flash) path with 16 heads.

Sharding: data-parallel over batch - core i handles batch element i; no
collectives, outputs are concatenated on the host.

Per-core strategy (fp8 DoubleRow matmuls at 2x PE throughput):
  - LN runs on transposed activations (d on partitions); stats are matmuls
    against an all-ones stationary so the sums land partition-replicated in
    PSUM; the normalize chain runs in bf16 on DVE and the gamma/beta apply
    runs on the Pool engine (tensor_scalar), writing xn directly in fp8.
  - QKV / V / PV / out-proj all run as fp8e4 DoubleRow matmuls with full
    128-wide stationaries: weights are pre-scaled by 32 on the host (power
    of two; folded back out via the softmax exp scale and the final output
    scale); each instruction contracts 2x128 d-coords at 0.5 cycles per
    output column.
  - scores^T[t,s] = K^T.T @ Q^T per head stay bf16 (K=64 contraction gains
    nothing from DoubleRow); exp runs on the scalar engine over [128, 1024]
    PSUM tiles (amortizing the fixed ACT access latency) with scale 1/8192
    and a -3 offset (cancels in softmax; keeps fp8 exp in range).
  - the softmax denominator comes from a DoubleRow matmul against an fp8
    all-ones stationary - its [64, N] output is the denominator replicated
    across 64 partitions, so the per-head normalize needs no broadcast.
  - PE emission: Q/K/V units are split by s-half so the first halves (plus
    warmup matmuls) keep the PE busy while LayerNorm finishes the second
    x chunk; per head pair the stream is qk(p+1) | scores(p) | pv+den(p-1)
    so the scalar engine's exp stream overlaps PE work throughout.
  - PSUM->SBUF copies alternate between DVE and the otherwise-idle Pool
    engine; residual + out_proj bias are pre-added on the host (bf16) and
    merged with one fused scalar_tensor_tensor: (psum * 2^-10) + resid.
"""

import numpy as np
import ml_dtypes

P = 128
D = 1024
H = 16
DH = 64
B = 8
S = 1024
LN_EPS = 1e-5
N_CORES = 8

_ND = D // P   # d tiles (8)
NS = S // P    # s tiles (8)
NCK = 512      # LN chunk / matmul moving width
WS = 32.0      # fp8 weight pre-scale (power of two)
EXP_SCALE = 0.125 / (WS * WS)   # 1/8192: folds 1/sqrt(dh) and the q/k scales
EXP_BIAS = -3.0                 # cancels in softmax; keeps fp8 exp in range
OUT_SCALE = 1.0 / (WS * WS)     # folds the v/out-proj weight scales back out

LAST_RESULTS = None
_NC_CACHE = {}


def _emit(tc, aps):
    from concourse import mybir

    nc = tc.nc
    f32 = mybir.dt.float32
    bf16 = mybir.dt.bfloat16
    fp8 = mybir.dt.float8e4
    FT = mybir.ActivationFunctionType
    OP = mybir.AluOpType
    DR = mybir.MatmulPerfMode.DoubleRow

    xT, resid, wqkt, wvt, woutt, binqk, binv, out = (
        aps["xt"], aps["resid"], aps["wqkt"], aps["wvt"], aps["woutt"],
        aps["binqk"], aps["binv"], aps["out"],
    )

    with tc.tile_pool(name="consts", bufs=1) as consts, \
         tc.tile_pool(name="acts", bufs=1) as acts, \
         tc.tile_pool(name="wpool", bufs=1) as wpool:

        # ---------- constants (DMAs issued after the first x chunk) ----------
        cvec = consts.tile([P, H + 1], f32)
        binqk_sb = cvec[:, 0:H]
        eps_sb = cvec[:, H:H + 1]
        nc.vector.memset(eps_sb, LN_EPS)
        ones_mat = consts.tile([P, P], bf16)
        nc.vector.memset(ones_mat, 1.0)
        ones8 = consts.tile([P, P], fp8)
        nc.vector.memset(ones8, 1.0)
        ones8_pl = ones8.rearrange("p (a m) -> p a m", a=2)  # [P, 2, 64]
        expb = consts.tile([P, 1], f32)
        nc.vector.memset(expb, EXP_BIAS)
        binv_bc = consts.tile([P, D], f32)

        # ---------- persistent activations ----------
        # staged out-proj partial (heads 0-7), with residual folded in
        xn8 = acts.tile([P, _ND, S], fp8)        # normalized x, transposed
        qkT = acts.tile([P, 2 * _ND, S], bf16)   # q tiles 0..7, k tiles 8..15
        v8 = acts.tile([P, NS, H, DH], fp8)      # v natural [t, h, dh]
        ctxT8 = acts.tile([P, _ND, S], fp8)      # normalized ctx^T (d on part)
        resid_sb = acts.tile([P, NS, D], bf16)   # x + out_proj_b, natural
        stage_sb = acts.tile([P, NS, D], bf16)

        # ---------- weights (fp8, pre-scaled by WS on host) ----------
        wqk_sb = wpool.tile([P, _ND, 2 * D], fp8)
        wv_sb = wpool.tile([P, _ND, D], fp8)
        wout_sb = wpool.tile([P, _ND, D], fp8)

        # ================= Phase 1: LayerNorm =================
        with tc.tile_pool(name="lnsb", bufs=1) as lnsb, \
             tc.tile_pool(name="lnrow", bufs=1) as lnrow, \
             tc.tile_pool(name="lntmp", bufs=2) as lntmp, \
             tc.tile_pool(name="lnps", bufs=1, space="PSUM") as lnps:
            xT_sb = lnsb.tile([P, _ND, S], bf16)
            sx_ps = lnps.tile([P, S], f32, tag="sx")
            sx2_ps = lnps.tile([P, S], f32, tag="sx2")
            # PE p-state warmup: dummy matmuls chain into the LN stats so the
            # clock is at full speed (and stays there) when real work arrives
            warm_ps = lnps.tile([P, P], f32, tag="warm")
            for _ in range(24):
                nc.tensor.matmul(warm_ps, lhsT=ones_mat, rhs=ones_mat,
                                 start=True, stop=True)
            xT_r = xT.rearrange("(a p) s -> p a s", p=P)
            for c in range(S // NCK):
                sl = slice(c * NCK, (c + 1) * NCK)
                for j in range(_ND):
                    nc.sync.dma_start(out=xT_sb[:, j, sl], in_=xT_r[:, j, sl])
                if c == 1:
                    # priority order: both x chunks, small consts, then the
                    # weights in first-use order; residual/out-proj last
                    nc.sync.dma_start(out=cvec[:, 0:H], in_=binqk)
                    nc.sync.dma_start(out=wqk_sb,
                                      in_=wqkt.rearrange("(a p) e -> p a e", p=P))
                    nc.sync.dma_start(out=wv_sb,
                                      in_=wvt.rearrange("(a p) e -> p a e", p=P))
                    nc.sync.dma_start(out=wout_sb,
                                      in_=woutt.rearrange("(a p) e -> p a e", p=P))
                    nc.sync.dma_start(out=resid_sb,
                                      in_=resid.rearrange("(st p) e -> p st e", p=P))
                    nc.gpsimd.dma_start(out=binv_bc,
                                        in_=binv[None, :].to_broadcast((P, D)))
                for j in range(_ND):
                    sq = lntmp.tile([P, NCK], bf16, tag="sq", bufs=4)
                    with nc.allow_low_precision(reason="x^2 for LN stats in bf16"):
                        if c == 0:
                            nc.vector.tensor_tensor(out=sq, in0=xT_sb[:, j, sl],
                                                    in1=xT_sb[:, j, sl], op=OP.mult)
                        else:
                            nc.scalar.square(out=sq, in_=xT_sb[:, j, sl])
                    nc.tensor.matmul(sx_ps[:, sl], lhsT=ones_mat, rhs=xT_sb[:, j, sl],
                                     start=(j == 0), stop=(j == _ND - 1))
                    nc.tensor.matmul(sx2_ps[:, sl], lhsT=ones_mat, rhs=sq,
                                     start=(j == 0), stop=(j == _ND - 1))

                with nc.allow_low_precision(reason="LN stats chain in bf16"):
                    mu_bc = lnrow.tile([P, NCK], bf16, tag="mu", bufs=2)
                    nc.vector.tensor_scalar_mul(mu_bc, sx_ps[:, sl], 1.0 / D)
                    var_bc = lnrow.tile([P, NCK], f32, tag="var", bufs=2)
                    nc.vector.tensor_scalar_mul(var_bc, sx2_ps[:, sl], 1.0 / D)
                    musq = lnrow.tile([P, NCK], bf16, tag="musq", bufs=2)
                    nc.vector.tensor_tensor(out=musq, in0=mu_bc, in1=mu_bc, op=OP.mult)
                    nc.vector.tensor_tensor(out=var_bc, in0=var_bc, in1=musq,
                                            op=OP.subtract)
                    std_bc = lnrow.tile([P, NCK], bf16, tag="std", bufs=2)
                    nc.scalar.activation(out=std_bc, in_=var_bc, func=FT.Sqrt,
                                         bias=eps_sb)
                    b_bc = lnrow.tile([P, NCK], bf16, tag="b", bufs=2)
                    nc.vector.reciprocal(out=b_bc, in_=std_bc)
                    mub_bc = std_bc
                    nc.vector.tensor_tensor(out=mub_bc, in0=mu_bc, in1=b_bc,
                                            op=OP.mult)

                    for j in range(_ND):
                        t = lntmp.tile([P, NCK], bf16, tag="nrm", bufs=4)
                        eng = nc.vector if j < 5 else nc.gpsimd
                        eng.tensor_tensor(out=t, in0=xT_sb[:, j, sl],
                                          in1=b_bc, op=OP.mult)
                        eng.tensor_tensor(out=xn8[:, j, sl], in0=t,
                                          in1=mub_bc, op=OP.subtract)

        # ============ Phases 2-4: projections + attention + out-proj ========
        with tc.tile_pool(name="expool", bufs=1) as expool, \
             tc.tile_pool(name="sidep", bufs=1) as sidep, \
             tc.tile_pool(name="mps", bufs=1, space="PSUM") as mps:

            def dr_matmul(ps_out, lhsT, rhs, start, stop):
                nc.tensor.matmul(ps_out, lhsT=lhsT, rhs=rhs, start=start,
                                 stop=stop, perf_mode=DR)

            def veng(i):
                return nc.vector if i % 2 == 0 else nc.gpsimd

            def emit_qk_half(et, half, on_act=False):
                # e-tile et (128 cols of q|k), s-half: one [128, 512] group
                ps = mps.tile([P, NCK], f32, tag="mm", bufs=2,
                              name=f"qk{et}_{half}")
                e0 = et * P
                for jp in range(_ND // 2):
                    for c2 in range(2):
                        sl = slice(half * NCK + c2 * 256,
                                   half * NCK + (c2 + 1) * 256)
                        dr_matmul(
                            ps[:, c2 * 256:(c2 + 1) * 256],
                            wqk_sb[:, 2 * jp:2 * jp + 2, e0:e0 + P],
                            xn8[:, 2 * jp:2 * jp + 2, sl],
                            start=(jp == 0 and c2 == 0),
                            stop=(jp == _ND // 2 - 1 and c2 == 1))
                sl = slice(half * NCK, (half + 1) * NCK)
                with nc.allow_low_precision(reason="qk to bf16"):
                    if on_act:
                        nc.scalar.activation(out=qkT[:, et, sl], in_=ps,
                                             func=FT.Identity,
                                             bias=binqk_sb[:, et:et + 1])
                    else:
                        nc.vector.tensor_scalar_add(qkT[:, et, sl], ps,
                                                    binqk_sb[:, et:et + 1])

            def emit_v_unit(st):
                # t-tile st: V natural [128 t, 512 e'] per e'-half
                for eh in range(2):
                    ps = mps.tile([P, NCK], f32, tag="mm", bufs=2,
                                  name=f"v{st}_{eh}")
                    t0 = st * P
                    for jp in range(_ND // 2):
                        for c2 in range(2):
                            sl = slice(eh * NCK + c2 * 256,
                                       eh * NCK + (c2 + 1) * 256)
                            dr_matmul(
                                ps[:, c2 * 256:(c2 + 1) * 256],
                                xn8[:, 2 * jp:2 * jp + 2, t0:t0 + P],
                                wv_sb[:, 2 * jp:2 * jp + 2, sl],
                                start=(jp == 0 and c2 == 0),
                                stop=(jp == _ND // 2 - 1 and c2 == 1))
                    with nc.allow_low_precision(reason="v to fp8"):
                        nc.vector.tensor_tensor(
                            out=v8[:, st, eh * 8:(eh + 1) * 8, :],
                            in0=ps.rearrange("p (h d) -> p h d", d=DH),
                            in1=binv_bc[:, eh * NCK:(eh + 1) * NCK]
                                .rearrange("p (h d) -> p h d", d=DH),
                            op=OP.add)

            def emit_scores(hp):
                # per head pair: scores^T into [128, 1024] psum tiles, then a
                # single wide exp (fp8 out) per (tt, idx)
                ex_t = expool.tile([P, 2, NS, S], fp8, tag="ex", bufs=2,
                                   name=f"ex{hp}")
                for tt in range(NS):
                    for idx in range(2):
                        base = idx * DH
                        ps = mps.tile([P, S], f32, tag="sc", bufs=2,
                                      name=f"sc{hp}_{tt}_{idx}")
                        for sh in range(2):
                            sl = slice(sh * NCK, (sh + 1) * NCK)
                            nc.tensor.matmul(
                                ps[:, sl],
                                lhsT=qkT[base:base + DH, 8 + hp, tt * P:(tt + 1) * P],
                                rhs=qkT[base:base + DH, hp, sl],
                                start=True, stop=True, tile_position=(base, 0))
                        with nc.allow_low_precision(reason="exp to fp8"):
                            nc.scalar.activation(out=ex_t[:, idx, tt, :],
                                                 in_=ps, func=FT.Exp,
                                                 scale=EXP_SCALE, bias=expb)
                return ex_t

            def emit_pvden(hp, ex_t):
                # PV + denominator (DoubleRow, planes = t-tile pairs), then
                # normalize ctx^T in fp8.  den rides the "mm" psum tag and
                # arrives replicated over 64 partitions (no broadcast needed).
                for sh in range(2):
                    for idx in range(2):
                        h = 2 * hp + idx
                        ctxps = mps.tile([DH, NCK], f32, tag="ctx", bufs=2,
                                         name=f"ctx{hp}_{sh}_{idx}")
                        denft = mps.tile([P, NCK], f32, tag="mm", bufs=2,
                                         name=f"den{hp}_{sh}_{idx}")
                        denps = denft[0:DH, :]
                        for ttp in range(NS // 2):
                            for c2 in range(2):
                                sl = slice(sh * NCK + c2 * 256,
                                           sh * NCK + (c2 + 1) * 256)
                                co = slice(c2 * 256, (c2 + 1) * 256)
                                st_ = (ttp == 0 and c2 == 0)
                                sp_ = (ttp == NS // 2 - 1 and c2 == 1)
                                dr_matmul(ctxps[:, co],
                                          v8[:, 2 * ttp:2 * ttp + 2, h, :],
                                          ex_t[:, idx, 2 * ttp:2 * ttp + 2, sl],
                                          start=st_, stop=sp_)
                                dr_matmul(denps[:, co], ones8_pl,
                                          ex_t[:, idx, 2 * ttp:2 * ttp + 2, sl],
                                          start=st_, stop=sp_)
                        sl = slice(sh * NCK, (sh + 1) * NCK)
                        rden = sidep.tile([DH, NCK], bf16, tag="rd", bufs=4,
                                          name=f"rd{hp}_{sh}_{idx}")
                        with nc.allow_low_precision(reason="denom in bf16"):
                            nc.vector.reciprocal(out=rden, in_=denps)
                            nc.vector.tensor_tensor(
                                out=ctxT8[idx * DH:(idx + 1) * DH, hp, sl],
                                in0=ctxps, in1=rden, op=OP.mult)

            def emit_outproj_a(sts):
                # heads 0-7 partial -> stage (residual folded in); runs in the
                # exp-bound era where PE and DVE have slack
                for st in sts:
                    ps = mps.tile([P, S], f32, tag="sc", bufs=2,
                                  name=f"opa{st}")
                    s0 = st * P
                    for eh in range(2):
                        for hpp in range(2):
                            for c2 in range(2):
                                sl = slice(eh * NCK + c2 * 256,
                                           eh * NCK + (c2 + 1) * 256)
                                dr_matmul(
                                    ps[:, sl],
                                    ctxT8[:, 2 * hpp:2 * hpp + 2, s0:s0 + P],
                                    wout_sb[:, 2 * hpp:2 * hpp + 2, sl],
                                    start=(hpp == 0 and c2 == 0),
                                    stop=(hpp == 1 and c2 == 1))
                    with nc.allow_low_precision(reason="stage in bf16"):
                        nc.vector.scalar_tensor_tensor(
                            out=stage_sb[:, st, :], in0=ps, scalar=OUT_SCALE,
                            in1=resid_sb[:, st, :], op0=OP.mult, op1=OP.add)

            def emit_outproj_b():
                for st in range(NS):
                    ps = mps.tile([P, S], f32, tag="sc", bufs=2,
                                  name=f"opb{st}")
                    s0 = st * P
                    for eh in range(2):
                        for hpp in range(2, _ND // 2):
                            for c2 in range(2):
                                sl = slice(eh * NCK + c2 * 256,
                                           eh * NCK + (c2 + 1) * 256)
                                dr_matmul(
                                    ps[:, sl],
                                    ctxT8[:, 2 * hpp:2 * hpp + 2, s0:s0 + P],
                                    wout_sb[:, 2 * hpp:2 * hpp + 2, sl],
                                    start=(hpp == 2 and c2 == 0),
                                    stop=(hpp == _ND // 2 - 1 and c2 == 1))
                    ot = sidep.tile([P, S], bf16, tag="ot", bufs=2,
                                    name=f"ot{st}")
                    ob = sidep.tile([P, S], bf16, tag="ob", bufs=2,
                                    name=f"ob{st}")
                    with nc.allow_low_precision(reason="out in bf16"):
                        nc.scalar.mul(ot, ps, OUT_SCALE)
                        nc.vector.tensor_tensor(out=ob, in0=ot,
                                                in1=stage_sb[:, st, :],
                                                op=OP.add)
                    nc.sync.dma_start(out=out[st * P:(st + 1) * P, :], in_=ob)

            # ---- interleaved emission ----
            # s-half-0 work first: runs while LayerNorm's second chunk is
            # still on DVE, keeping the PE busy
            # pair-0 q/k first (s-half-0 while LN's second chunk runs),
            # with v fills riding the xn-c1 wait
            emit_qk_half(0, 0, on_act=True)
            emit_qk_half(8, 0, on_act=True)
            for st in range(NS // 2):
                emit_v_unit(st)
            emit_qk_half(0, 1, on_act=True)
            emit_qk_half(8, 1, on_act=True)
            ex_prev = emit_scores(0)
            for half in range(2):
                emit_qk_half(1, half)
                emit_qk_half(9, half)
            ex_cur = emit_scores(1)
            for st in range(NS // 2, NS):
                emit_v_unit(st)
            emit_pvden(0, ex_prev)
            ex_prev = ex_cur
            for p in range(2, H // 2):
                for half in range(2):
                    emit_qk_half(p, half)
                    emit_qk_half(8 + p, half)
                ex_cur = emit_scores(p)
                emit_pvden(p - 1, ex_prev)
                ex_prev = ex_cur
                if p == 5:
                    emit_outproj_a(range(NS // 2))
                elif p == 6:
                    emit_outproj_a(range(NS // 2, NS))
            emit_pvden(H // 2 - 1, ex_prev)
            emit_outproj_b()


def build_nc():
    import concourse.bacc as bacc
    import concourse.tile as tile
    from concourse import mybir

    f32 = mybir.dt.float32
    bf16 = mybir.dt.bfloat16
    fp8 = mybir.dt.float8e4

    nc = bacc.Bacc("TRN2", target_bir_lowering=False, debug=False)
    aps = {
        "xt": nc.dram_tensor("xt", [D, S], bf16, kind="ExternalInput").ap(),
        "resid": nc.dram_tensor("resid", [S, D], bf16, kind="ExternalInput").ap(),
        "wqkt": nc.dram_tensor("wqkt", [D, 2 * D], fp8, kind="ExternalInput").ap(),
        "wvt": nc.dram_tensor("wvt", [D, D], fp8, kind="ExternalInput").ap(),
        "woutt": nc.dram_tensor("woutt", [D, D], fp8, kind="ExternalInput").ap(),
        "binqk": nc.dram_tensor("binqk", [P, H], f32, kind="ExternalInput").ap(),
        "binv": nc.dram_tensor("binv", [D], f32, kind="ExternalInput").ap(),
        "out": nc.dram_tensor("out", [S, D], bf16, kind="ExternalOutput").ap(),
    }
    with tile.TileContext(nc) as tc:
        _emit(tc, aps)
    nc.compile()
    return nc


def prep_inputs(x, ln_gamma, ln_beta, in_proj_w, in_proj_b, out_proj_w, out_proj_b,
                n_cores=N_CORES):
    bf = ml_dtypes.bfloat16
    f8 = ml_dtypes.float8_e4m3
    f32c = lambda a: np.ascontiguousarray(a, dtype=np.float32)
    win = np.asarray(in_proj_w, np.float32)
    g = np.asarray(ln_gamma, np.float32)
    bt = np.asarray(ln_beta, np.float32)
    bin_ = np.asarray(in_proj_b, np.float32)
    wing = win * g[None, :]          # gamma folded into in-proj columns
    binf = bin_ + win @ bt           # beta folded into the in-proj biases
    shared = {
        "wqkt": np.ascontiguousarray((wing[:2 * D] * WS).T).astype(f8),
        "wvt": np.ascontiguousarray((wing[2 * D:] * WS).T).astype(f8),
        "woutt": np.ascontiguousarray(np.asarray(out_proj_w, np.float32).T * WS).astype(f8),
        "binqk": f32c((binf[:2 * D] * WS).reshape(H, P).T),
        "binv": f32c(binf[2 * D:] * WS),
    }
    bout = np.asarray(out_proj_b, np.float32)
    in_maps = []
    for i in range(n_cores):
        xi = np.asarray(x[i], np.float32)
        m = dict(shared)
        m["xt"] = np.ascontiguousarray(xi.T).astype(bf)
        m["resid"] = np.ascontiguousarray(xi + bout).astype(bf)
        in_maps.append(m)
    return in_maps


def kernel(x, ln_gamma, ln_beta, in_proj_w, in_proj_b, out_proj_w, out_proj_b):
    global LAST_RESULTS
    from concourse import bass_utils

    if "nc" not in _NC_CACHE:
        _NC_CACHE["nc"] = build_nc()
    nc = _NC_CACHE["nc"]

    in_maps = prep_inputs(x, ln_gamma, ln_beta, in_proj_w, in_proj_b,
                          out_proj_w, out_proj_b)
    res = bass_utils.run_bass_kernel_spmd(nc, in_maps, core_ids=list(range(N_CORES)))
    LAST_RESULTS = res
    out = np.stack([r["out"] for r in res.results], axis=0)
    return np.ascontiguousarray(out, dtype=np.float32)


# revision 16
# speedup vs baseline: 1.0227x; 1.0227x over previous
"""Self-contained Trainium2 Bass kernel: pre-LN multi-head attention block.

Computes, for x [B=8, S=1024, D=1024] (fp32) and packed attention weights:
    out = x + out_proj(MHA(LayerNorm(x)))
matching torch nn.MultiheadAttention's explicit (non-flash) path with 16 heads.

Sharding: data-parallel over batch - core i handles batch element i; no
collectives, outputs are concatenated on the host.

Per-core strategy (fp8 DoubleRow matmuls at 2x PE throughput):
  - LN runs on transposed activations (d on partitions); stats are matmuls
    against an all-ones stationary so the sums land partition-replicated in
    PSUM; the normalize chain runs in bf16 on DVE and the gamma/beta apply
    runs on the Pool engine (tensor_scalar), writing xn directly in fp8.
  - QKV / V / PV / out-proj all run as fp8e4 DoubleRow matmuls with full
    128-wide stationaries: weights are pre-scaled by 32 on the host (power
    of two; folded back out via the softmax exp scale and the final output
    scale); each instruction contracts 2x128 d-coords at 0.5 cycles per
    output column.
  - scores^T[t,s] = K^T.T @ Q^T per head stay bf16 (K=64 contraction gains
    nothing from DoubleRow); exp runs on the scalar engine over [128, 1024]
    PSUM tiles (amortizing the fixed ACT access latency) with scale 1/8192
    and a -3 offset (cancels in softmax; keeps fp8 exp in range).
  - the softmax denominator comes from a DoubleRow matmul against an fp8
    all-ones stationary - its [64, N] output is the denominator replicated
    across 64 partitions, so the per-head normalize needs no broadcast.
  - PE emission: Q/K/V units are split by s-half so the first halves (plus
    warmup matmuls) keep the PE busy while LayerNorm finishes the second
    x chunk; per head pair the stream is qk(p+1) | scores(p) | pv+den(p-1)
    so the scalar engine's exp stream overlaps PE work throughout.
  - PSUM->SBUF copies alternate between DVE and the otherwise-idle Pool
    engine; residual + out_proj bias are pre-added on the host (bf16) and
    merged with one fused scalar_tensor_tensor: (psum * 2^-10) + resid.
"""

import numpy as np
import ml_dtypes

P = 128
D = 1024
H = 16
DH = 64
B = 8
S = 1024
LN_EPS = 1e-5
N_CORES = 8

_ND = D // P   # d tiles (8)
NS = S // P    # s tiles (8)
NCK = 512      # LN chunk / matmul moving width
WS = 32.0      # fp8 weight pre-scale (power of two)
EXP_SCALE = 0.125 / (WS * WS)   # 1/8192: folds 1/sqrt(dh) and the q/k scales
EXP_BIAS = -3.0                 # cancels in softmax; keeps fp8 exp in range
OUT_SCALE = 1.0 / (WS * WS)     # folds the v/out-proj weight scales back out

LAST_RESULTS = None
_NC_CACHE = {}


def _emit(tc, aps):
    from concourse import mybir

    nc = tc.nc
    f32 = mybir.dt.float32
    bf16 = mybir.dt.bfloat16
    fp8 = mybir.dt.float8e4
    FT = mybir.ActivationFunctionType
    OP = mybir.AluOpType
    DR = mybir.MatmulPerfMode.DoubleRow

    xT, resid, wqkt, wvt, woutt, binqk, binv, out = (
        aps["xt"], aps["resid"], aps["wqkt"], aps["wvt"], aps["woutt"],
        aps["binqk"], aps["binv"], aps["out"],
    )

    with tc.tile_pool(name="consts", bufs=1) as consts, \
         tc.tile_pool(name="acts", bufs=1) as acts, \
         tc.tile_pool(name="wpool", bufs=1) as wpool:

        # ---------- constants (DMAs issued after the first x chunk) ----------
        cvec = consts.tile([P, H + 1], f32)
        binqk_sb = cvec[:, 0:H]
        eps_sb = cvec[:, H:H + 1]
        nc.vector.memset(eps_sb, LN_EPS)
        ones_mat = consts.tile([P, P], bf16)
        nc.vector.memset(ones_mat, 1.0)
        ones8 = consts.tile([P, P], fp8)
        nc.vector.memset(ones8, 1.0)
        ones8_pl = ones8.rearrange("p (a m) -> p a m", a=2)  # [P, 2, 64]
        expb = consts.tile([P, 1], f32)
        nc.vector.memset(expb, EXP_BIAS)
        binv_bc = consts.tile([P, D], f32)

        # ---------- persistent activations ----------
        # staged out-proj partial (heads 0-7), with residual folded in
        xn8 = acts.tile([P, _ND, S], fp8)        # normalized x, transposed
        qkT = acts.tile([P, 2 * _ND, S], bf16)   # q tiles 0..7, k tiles 8..15
        v8 = acts.tile([P, NS, H, DH], fp8)      # v natural [t, h, dh]
        ctxT8 = acts.tile([P, _ND, S], fp8)      # normalized ctx^T (d on part)
        resid_sb = acts.tile([P, NS, D], bf16)   # x + out_proj_b, natural
        stage_sb = acts.tile([P, NS, D], bf16)

        # ---------- weights (fp8, pre-scaled by WS on host) ----------
        wqk_sb = wpool.tile([P, _ND, 2 * D], fp8)
        wv_sb = wpool.tile([P, _ND, D], fp8)
        wout_sb = wpool.tile([P, _ND, D], fp8)

        # ================= Phase 1: LayerNorm =================
        with tc.tile_pool(name="lnsb", bufs=1) as lnsb, \
             tc.tile_pool(name="lnrow", bufs=1) as lnrow, \
             tc.tile_pool(name="lntmp", bufs=2) as lntmp, \
             tc.tile_pool(name="lnps", bufs=1, space="PSUM") as lnps:
            xT_sb = lnsb.tile([P, _ND, S], bf16)
            sx_ps = lnps.tile([P, S], f32, tag="sx")
            sx2_ps = lnps.tile([P, S], f32, tag="sx2")
            # PE p-state warmup: dummy matmuls chain into the LN stats so the
            # clock is at full speed (and stays there) when real work arrives
            warm_ps = lnps.tile([P, P], f32, tag="warm")
            for _ in range(24):
                nc.tensor.matmul(warm_ps, lhsT=ones_mat, rhs=ones_mat,
                                 start=True, stop=True)
            xT_r = xT.rearrange("(a p) s -> p a s", p=P)
            # stage the DMAs for both chunks, then both stats blocks (sq on
            # DVE for chunk 0, on ACT for chunk 1 so neither engine queues
            # behind the other chunk's chain), then chain+normalize per chunk
            for c in range(S // NCK):
                sl = slice(c * NCK, (c + 1) * NCK)
                for j in range(_ND):
                    nc.sync.dma_start(out=xT_sb[:, j, sl], in_=xT_r[:, j, sl])
                if c == 0:
                    nc.sync.dma_start(out=cvec[:, 0:H], in_=binqk)
            nc.sync.dma_start(out=wqk_sb,
                              in_=wqkt.rearrange("(a p) e -> p a e", p=P))
            nc.sync.dma_start(out=wv_sb,
                              in_=wvt.rearrange("(a p) e -> p a e", p=P))
            nc.sync.dma_start(out=wout_sb,
                              in_=woutt.rearrange("(a p) e -> p a e", p=P))
            nc.sync.dma_start(out=resid_sb,
                              in_=resid.rearrange("(st p) e -> p st e", p=P))
            nc.gpsimd.dma_start(out=binv_bc,
                                in_=binv[None, :].to_broadcast((P, D)))
            sqs = {}
            for c in range(S // NCK):
                sl = slice(c * NCK, (c + 1) * NCK)
                for j in range(_ND):
                    sq = lntmp.tile([P, NCK], bf16, tag=f"sq{c}", bufs=8,
                                    name=f"sq{c}_{j}")
                    sqs[(c, j)] = sq
                    with nc.allow_low_precision(reason="x^2 for LN stats"):
                        if c == 0:
                            nc.vector.tensor_tensor(out=sq, in0=xT_sb[:, j, sl],
                                                    in1=xT_sb[:, j, sl], op=OP.mult)
                        else:
                            nc.scalar.square(out=sq, in_=xT_sb[:, j, sl])
            for c in range(S // NCK):
                sl = slice(c * NCK, (c + 1) * NCK)
                for j in range(_ND):
                    nc.tensor.matmul(sx_ps[:, sl], lhsT=ones_mat,
                                     rhs=xT_sb[:, j, sl],
                                     start=(j == 0), stop=(j == _ND - 1))
                    nc.tensor.matmul(sx2_ps[:, sl], lhsT=ones_mat, rhs=sqs[(c, j)],
                                     start=(j == 0), stop=(j == _ND - 1))
            for c in range(S // NCK):
                sl = slice(c * NCK, (c + 1) * NCK)
                with nc.allow_low_precision(reason="LN stats chain in bf16"):
                    mu_bc = lnrow.tile([P, NCK], bf16, tag="mu", bufs=2)
                    nc.vector.tensor_scalar_mul(mu_bc, sx_ps[:, sl], 1.0 / D)
                    var_bc = lnrow.tile([P, NCK], f32, tag="var", bufs=2)
                    nc.vector.tensor_scalar_mul(var_bc, sx2_ps[:, sl], 1.0 / D)
                    musq = lnrow.tile([P, NCK], bf16, tag="musq", bufs=2)
                    nc.vector.tensor_tensor(out=musq, in0=mu_bc, in1=mu_bc,
                                            op=OP.mult)
                    nc.vector.tensor_tensor(out=var_bc, in0=var_bc, in1=musq,
                                            op=OP.subtract)
                    std_bc = lnrow.tile([P, NCK], bf16, tag="std", bufs=2)
                    nc.scalar.activation(out=std_bc, in_=var_bc, func=FT.Sqrt,
                                         bias=eps_sb)
                    b_bc = lnrow.tile([P, NCK], bf16, tag="b", bufs=2)
                    nc.vector.reciprocal(out=b_bc, in_=std_bc)
                    mub_bc = std_bc
                    nc.vector.tensor_tensor(out=mub_bc, in0=mu_bc, in1=b_bc,
                                            op=OP.mult)
                    for j in range(_ND):
                        t = lntmp.tile([P, NCK], bf16, tag="nrm", bufs=4)
                        eng = nc.vector if j < 5 else nc.gpsimd
                        eng.tensor_tensor(out=t, in0=xT_sb[:, j, sl],
                                          in1=b_bc, op=OP.mult)
                        eng.tensor_tensor(out=xn8[:, j, sl], in0=t,
                                          in1=mub_bc, op=OP.subtract)

        # ============ Phases 2-4: projections + attention + out-proj ========
        with tc.tile_pool(name="expool", bufs=1) as expool, \
             tc.tile_pool(name="sidep", bufs=1) as sidep, \
             tc.tile_pool(name="mps", bufs=1, space="PSUM") as mps:

            def dr_matmul(ps_out, lhsT, rhs, start, stop):
                nc.tensor.matmul(ps_out, lhsT=lhsT, rhs=rhs, start=start,
                                 stop=stop, perf_mode=DR)

            def veng(i):
                return nc.vector if i % 2 == 0 else nc.gpsimd

            def emit_qk_half(et, half, on_act=False):
                # e-tile et (128 cols of q|k), s-half: one [128, 512] group
                ps = mps.tile([P, NCK], f32, tag="mm", bufs=2,
                              name=f"qk{et}_{half}")
                e0 = et * P
                for jp in range(_ND // 2):
                    for c2 in range(2):
                        sl = slice(half * NCK + c2 * 256,
                                   half * NCK + (c2 + 1) * 256)
                        dr_matmul(
                            ps[:, c2 * 256:(c2 + 1) * 256],
                            wqk_sb[:, 2 * jp:2 * jp + 2, e0:e0 + P],
                            xn8[:, 2 * jp:2 * jp + 2, sl],
                            start=(jp == 0 and c2 == 0),
                            stop=(jp == _ND // 2 - 1 and c2 == 1))
                sl = slice(half * NCK, (half + 1) * NCK)
                with nc.allow_low_precision(reason="qk to bf16"):
                    if on_act:
                        nc.scalar.activation(out=qkT[:, et, sl], in_=ps,
                                             func=FT.Identity,
                                             bias=binqk_sb[:, et:et + 1])
                    else:
                        nc.vector.tensor_scalar_add(qkT[:, et, sl], ps,
                                                    binqk_sb[:, et:et + 1])

            def emit_v_unit(st):
                # t-tile st: V natural [128 t, 512 e'] per e'-half
                for eh in range(2):
                    ps = mps.tile([P, NCK], f32, tag="mm", bufs=2,
                                  name=f"v{st}_{eh}")
                    t0 = st * P
                    for jp in range(_ND // 2):
                        for c2 in range(2):
                            sl = slice(eh * NCK + c2 * 256,
                                       eh * NCK + (c2 + 1) * 256)
                            dr_matmul(
                                ps[:, c2 * 256:(c2 + 1) * 256],
                                xn8[:, 2 * jp:2 * jp + 2, t0:t0 + P],
                                wv_sb[:, 2 * jp:2 * jp + 2, sl],
                                start=(jp == 0 and c2 == 0),
                                stop=(jp == _ND // 2 - 1 and c2 == 1))
                    with nc.allow_low_precision(reason="v to fp8"):
                        nc.vector.tensor_tensor(
                            out=v8[:, st, eh * 8:(eh + 1) * 8, :],
                            in0=ps.rearrange("p (h d) -> p h d", d=DH),
                            in1=binv_bc[:, eh * NCK:(eh + 1) * NCK]
                                .rearrange("p (h d) -> p h d", d=DH),
                            op=OP.add)

            def emit_scores(hp):
                # per head pair: scores^T into [128, 1024] psum tiles, then a
                # single wide exp (fp8 out) per (tt, idx)
                ex_t = expool.tile([P, 2, NS, S], fp8, tag="ex", bufs=2,
                                   name=f"ex{hp}")
                for tt in range(NS):
                    for idx in range(2):
                        base = idx * DH
                        ps = mps.tile([P, S], f32, tag="sc", bufs=2,
                                      name=f"sc{hp}_{tt}_{idx}")
                        for sh in range(2):
                            sl = slice(sh * NCK, (sh + 1) * NCK)
                            nc.tensor.matmul(
                                ps[:, sl],
                                lhsT=qkT[base:base + DH, 8 + hp, tt * P:(tt + 1) * P],
                                rhs=qkT[base:base + DH, hp, sl],
                                start=True, stop=True, tile_position=(base, 0))
                        with nc.allow_low_precision(reason="exp to fp8"):
                            nc.scalar.activation(out=ex_t[:, idx, tt, :],
                                                 in_=ps, func=FT.Exp,
                                                 scale=EXP_SCALE, bias=expb)
                return ex_t

            def emit_pvden(hp, ex_t):
                # PV + denominator (DoubleRow, planes = t-tile pairs), then
                # normalize ctx^T in fp8.  den rides the "mm" psum tag and
                # arrives replicated over 64 partitions (no broadcast needed).
                for sh in range(2):
                    for idx in range(2):
                        h = 2 * hp + idx
                        ctxps = mps.tile([DH, NCK], f32, tag="ctx", bufs=2,
                                         name=f"ctx{hp}_{sh}_{idx}")
                        denft = mps.tile([P, NCK], f32, tag="mm", bufs=2,
                                         name=f"den{hp}_{sh}_{idx}")
                        denps = denft[0:DH, :]
                        for ttp in range(NS // 2):
                            for c2 in range(2):
                                sl = slice(sh * NCK + c2 * 256,
                                           sh * NCK + (c2 + 1) * 256)
                                co = slice(c2 * 256, (c2 + 1) * 256)
                                st_ = (ttp == 0 and c2 == 0)
                                sp_ = (ttp == NS // 2 - 1 and c2 == 1)
                                dr_matmul(ctxps[:, co],
                                          v8[:, 2 * ttp:2 * ttp + 2, h, :],
                                          ex_t[:, idx, 2 * ttp:2 * ttp + 2, sl],
                                          start=st_, stop=sp_)
                                dr_matmul(denps[:, co], ones8_pl,
                                          ex_t[:, idx, 2 * ttp:2 * ttp + 2, sl],
                                          start=st_, stop=sp_)
                        sl = slice(sh * NCK, (sh + 1) * NCK)
                        rden = sidep.tile([DH, NCK], bf16, tag="rd", bufs=4,
                                          name=f"rd{hp}_{sh}_{idx}")
                        with nc.allow_low_precision(reason="denom in bf16"):
                            nc.vector.reciprocal(out=rden, in_=denps)
                            nc.vector.tensor_tensor(
                                out=ctxT8[idx * DH:(idx + 1) * DH, hp, sl],
                                in0=ctxps, in1=rden, op=OP.mult)

            def emit_outproj_a(sts):
                # heads 0-7 partial -> stage (residual folded in); runs in the
                # exp-bound era where PE and DVE have slack
                for st in sts:
                    s0 = st * P
                    for eh in range(2):
                        ps = mps.tile([P, NCK], f32, tag="mm", bufs=2,
                                      name=f"opa{st}_{eh}")
                        for hpp in range(2):
                            for c2 in range(2):
                                sl = slice(eh * NCK + c2 * 256,
                                           eh * NCK + (c2 + 1) * 256)
                                dr_matmul(
                                    ps[:, c2 * 256:(c2 + 1) * 256],
                                    ctxT8[:, 2 * hpp:2 * hpp + 2, s0:s0 + P],
                                    wout_sb[:, 2 * hpp:2 * hpp + 2, sl],
                                    start=(hpp == 0 and c2 == 0),
                                    stop=(hpp == 1 and c2 == 1))
                        sl = slice(eh * NCK, (eh + 1) * NCK)
                        with nc.allow_low_precision(reason="stage in bf16"):
                            nc.vector.scalar_tensor_tensor(
                                out=stage_sb[:, st, sl], in0=ps, scalar=OUT_SCALE,
                                in1=resid_sb[:, st, sl], op0=OP.mult, op1=OP.add)

            def emit_outproj_b():
                for st in range(NS):
                    ps = mps.tile([P, S], f32, tag="sc", bufs=2,
                                  name=f"opb{st}")
                    s0 = st * P
                    for eh in range(2):
                        for hpp in range(2, _ND // 2):
                            for c2 in range(2):
                                sl = slice(eh * NCK + c2 * 256,
                                           eh * NCK + (c2 + 1) * 256)
                                dr_matmul(
                                    ps[:, sl],
                                    ctxT8[:, 2 * hpp:2 * hpp + 2, s0:s0 + P],
                                    wout_sb[:, 2 * hpp:2 * hpp + 2, sl],
                                    start=(hpp == 2 and c2 == 0),
                                    stop=(hpp == _ND // 2 - 1 and c2 == 1))
                    ot = sidep.tile([P, S], bf16, tag="ot", bufs=2,
                                    name=f"ot{st}")
                    ob = sidep.tile([P, S], bf16, tag="ob", bufs=2,
                                    name=f"ob{st}")
                    with nc.allow_low_precision(reason="out in bf16"):
                        nc.scalar.mul(ot, ps, OUT_SCALE)
                        nc.vector.tensor_tensor(out=ob, in0=ot,
                                                in1=stage_sb[:, st, :],
                                                op=OP.add)
                    nc.sync.dma_start(out=out[st * P:(st + 1) * P, :], in_=ob)

            # ---- interleaved emission ----
            # s-half-0 work first: runs while LayerNorm's second chunk is
            # still on DVE, keeping the PE busy
            # pair-0 q/k first (s-half-0 while LN's second chunk runs),
            # with v fills riding the xn-c1 wait
            emit_qk_half(0, 0, on_act=True)
            emit_qk_half(8, 0, on_act=True)
            for st in range(NS // 2):
                emit_v_unit(st)
            emit_qk_half(0, 1, on_act=True)
            emit_qk_half(8, 1, on_act=True)
            ex_prev = emit_scores(0)
            for half in range(2):
                emit_qk_half(1, half)
                emit_qk_half(9, half)
            ex_cur = emit_scores(1)
            for st in range(NS // 2, NS):
                emit_v_unit(st)
            emit_pvden(0, ex_prev)
            ex_prev = ex_cur
            for p in range(2, H // 2):
                for half in range(2):
                    emit_qk_half(p, half)
                    emit_qk_half(8 + p, half)
                ex_cur = emit_scores(p)
                emit_pvden(p - 1, ex_prev)
                ex_prev = ex_cur
                if p >= 4:
                    emit_outproj_a(range(2 * (p - 4), 2 * (p - 3)))
            emit_pvden(H // 2 - 1, ex_prev)
            emit_outproj_b()


def build_nc():
    import concourse.bacc as bacc
    import concourse.tile as tile
    from concourse import mybir

    f32 = mybir.dt.float32
    bf16 = mybir.dt.bfloat16
    fp8 = mybir.dt.float8e4

    nc = bacc.Bacc("TRN2", target_bir_lowering=False, debug=False)
    aps = {
        "xt": nc.dram_tensor("xt", [D, S], bf16, kind="ExternalInput").ap(),
        "resid": nc.dram_tensor("resid", [S, D], bf16, kind="ExternalInput").ap(),
        "wqkt": nc.dram_tensor("wqkt", [D, 2 * D], fp8, kind="ExternalInput").ap(),
        "wvt": nc.dram_tensor("wvt", [D, D], fp8, kind="ExternalInput").ap(),
        "woutt": nc.dram_tensor("woutt", [D, D], fp8, kind="ExternalInput").ap(),
        "binqk": nc.dram_tensor("binqk", [P, H], f32, kind="ExternalInput").ap(),
        "binv": nc.dram_tensor("binv", [D], f32, kind="ExternalInput").ap(),
        "out": nc.dram_tensor("out", [S, D], bf16, kind="ExternalOutput").ap(),
    }
    with tile.TileContext(nc) as tc:
        _emit(tc, aps)
    nc.compile()
    return nc


def prep_inputs(x, ln_gamma, ln_beta, in_proj_w, in_proj_b, out_proj_w, out_proj_b,
                n_cores=N_CORES):
    bf = ml_dtypes.bfloat16
    f8 = ml_dtypes.float8_e4m3
    f32c = lambda a: np.ascontiguousarray(a, dtype=np.float32)
    win = np.asarray(in_proj_w, np.float32)
    g = np.asarray(ln_gamma, np.float32)
    bt = np.asarray(ln_beta, np.float32)
    bin_ = np.asarray(in_proj_b, np.float32)
    wing = win * g[None, :]          # gamma folded into in-proj columns
    binf = bin_ + win @ bt           # beta folded into the in-proj biases
    shared = {
        "wqkt": np.ascontiguousarray((wing[:2 * D] * WS).T).astype(f8),
        "wvt": np.ascontiguousarray((wing[2 * D:] * WS).T).astype(f8),
        "woutt": np.ascontiguousarray(np.asarray(out_proj_w, np.float32).T * WS).astype(f8),
        "binqk": f32c((binf[:2 * D] * WS).reshape(H, P).T),
        "binv": f32c(binf[2 * D:] * WS),
    }
    bout = np.asarray(out_proj_b, np.float32)
    in_maps = []
    for i in range(n_cores):
        xi = np.asarray(x[i], np.float32)
        m = dict(shared)
        m["xt"] = np.ascontiguousarray(xi.T).astype(bf)
        m["resid"] = np.ascontiguousarray(xi + bout).astype(bf)
        in_maps.append(m)
    return in_maps


def kernel(x, ln_gamma, ln_beta, in_proj_w, in_proj_b, out_proj_w, out_proj_b):
    global LAST_RESULTS
    from concourse import bass_utils

    if "nc" not in _NC_CACHE:
        _NC_CACHE["nc"] = build_nc()
    nc = _NC_CACHE["nc"]

    in_maps = prep_inputs(x, ln_gamma, ln_beta, in_proj_w, in_proj_b,
                          out_proj_w, out_proj_b)
    res = bass_utils.run_bass_kernel_spmd(nc, in_maps, core_ids=list(range(N_CORES)))
    LAST_RESULTS = res
    out = np.stack([r["out"] for r in res.results], axis=0)
    return np.ascontiguousarray(out, dtype=np.float32)


# revision 19
# speedup vs baseline: 1.0419x; 1.0188x over previous
"""Self-contained Trainium2 Bass kernel: pre-LN multi-head attention block.

Computes, for x [B=8, S=1024, D=1024] (fp32) and packed attention weights:
    out = x + out_proj(MHA(LayerNorm(x)))
matching torch nn.MultiheadAttention's explicit (non-flash) path with 16 heads.

Sharding: data-parallel over batch - core i handles batch element i; no
collectives, outputs are concatenated on the host.

Per-core strategy (fp8 DoubleRow matmuls at 2x PE throughput):
  - LN runs on transposed activations (d on partitions); stats are matmuls
    against an all-ones stationary so the sums land partition-replicated in
    PSUM; the normalize chain runs in bf16 on DVE and the gamma/beta apply
    runs on the Pool engine (tensor_scalar), writing xn directly in fp8.
  - QKV / V / PV / out-proj all run as fp8e4 DoubleRow matmuls with full
    128-wide stationaries: weights are pre-scaled by 32 on the host (power
    of two; folded back out via the softmax exp scale and the final output
    scale); each instruction contracts 2x128 d-coords at 0.5 cycles per
    output column.
  - scores^T[t,s] = K^T.T @ Q^T per head stay bf16 (K=64 contraction gains
    nothing from DoubleRow); exp runs on the scalar engine over [128, 1024]
    PSUM tiles (amortizing the fixed ACT access latency) with scale 1/8192
    and a -3 offset (cancels in softmax; keeps fp8 exp in range).
  - the softmax denominator comes from a DoubleRow matmul against an fp8
    all-ones stationary - its [64, N] output is the denominator replicated
    across 64 partitions, so the per-head normalize needs no broadcast.
  - PE emission: Q/K/V units are split by s-half so the first halves (plus
    warmup matmuls) keep the PE busy while LayerNorm finishes the second
    x chunk; per head pair the stream is qk(p+1) | scores(p) | pv+den(p-1)
    so the scalar engine's exp stream overlaps PE work throughout.
  - PSUM->SBUF copies alternate between DVE and the otherwise-idle Pool
    engine; residual + out_proj bias are pre-added on the host (bf16) and
    merged with one fused scalar_tensor_tensor: (psum * 2^-10) + resid.
"""

import numpy as np
import ml_dtypes

P = 128
D = 1024
H = 16
DH = 64
B = 8
S = 1024
LN_EPS = 1e-5
N_CORES = 8

_ND = D // P   # d tiles (8)
NS = S // P    # s tiles (8)
NCK = 512      # LN chunk / matmul moving width
WS = 32.0      # fp8 weight pre-scale (power of two)
EXP_SCALE = 0.125 / (WS * WS)   # 1/8192: folds 1/sqrt(dh) and the q/k scales
EXP_BIAS = -3.0                 # cancels in softmax; keeps fp8 exp in range
OUT_SCALE = 1.0 / (WS * WS)     # folds the v/out-proj weight scales back out

LAST_RESULTS = None
_NC_CACHE = {}


def _emit(tc, aps):
    from concourse import mybir

    nc = tc.nc
    f32 = mybir.dt.float32
    bf16 = mybir.dt.bfloat16
    fp8 = mybir.dt.float8e4
    FT = mybir.ActivationFunctionType
    OP = mybir.AluOpType
    DR = mybir.MatmulPerfMode.DoubleRow

    xT, resid, wqkt, wvt, woutt, binqk, binv, out = (
        aps["xt"], aps["resid"], aps["wqkt"], aps["wvt"], aps["woutt"],
        aps["binqk"], aps["binv"], aps["out"],
    )

    with tc.tile_pool(name="consts", bufs=1) as consts, \
         tc.tile_pool(name="acts", bufs=1) as acts, \
         tc.tile_pool(name="wpool", bufs=1) as wpool:

        # ---------- constants (DMAs issued after the first x chunk) ----------
        cvec = consts.tile([P, H + 1], f32)
        binqk_sb = cvec[:, 0:H]
        eps_sb = cvec[:, H:H + 1]
        nc.vector.memset(eps_sb, LN_EPS)
        ones_mat = consts.tile([P, P], bf16)
        nc.vector.memset(ones_mat, 1.0)
        ones8 = consts.tile([P, P], fp8)
        nc.vector.memset(ones8, 1.0)
        ones8_pl = ones8.rearrange("p (a m) -> p a m", a=2)  # [P, 2, 64]
        expb = consts.tile([P, 1], f32)
        nc.vector.memset(expb, EXP_BIAS)
        binv_bc = consts.tile([P, D], f32)

        # ---------- persistent activations ----------
        # staged out-proj partial (heads 0-7), with residual folded in
        xn8 = acts.tile([P, _ND, S], fp8)        # normalized x, transposed
        qkT = acts.tile([P, 2 * _ND, S], bf16)   # q tiles 0..7, k tiles 8..15
        v8 = acts.tile([P, NS, H, DH], fp8)      # v natural [t, h, dh]
        ctxT8 = acts.tile([P, _ND, S], fp8)      # normalized ctx^T (d on part)
        resid_sb = acts.tile([P, NS, D], bf16)   # x + out_proj_b, natural
        stage_sb = acts.tile([P, NS, D], bf16)

        # ---------- weights (fp8, pre-scaled by WS on host) ----------
        wqk_sb = wpool.tile([P, _ND, 2 * D], fp8)
        wv_sb = wpool.tile([P, _ND, D], fp8)
        wout_sb = wpool.tile([P, _ND, D], fp8)

        # ================= Phase 1: LayerNorm =================
        with tc.tile_pool(name="lnsb", bufs=1) as lnsb, \
             tc.tile_pool(name="lnrow", bufs=1) as lnrow, \
             tc.tile_pool(name="lntmp", bufs=2) as lntmp, \
             tc.tile_pool(name="lnps", bufs=1, space="PSUM") as lnps:
            xT_sb = lnsb.tile([P, _ND, S], bf16)
            sx_ps = lnps.tile([P, S], f32, tag="sx")
            sx2_ps = lnps.tile([P, S], f32, tag="sx2")
            # PE p-state warmup: dummy matmuls chain into the LN stats so the
            # clock is at full speed (and stays there) when real work arrives
            warm_ps = lnps.tile([P, P], f32, tag="warm")
            for _ in range(24):
                nc.tensor.matmul(warm_ps, lhsT=ones_mat, rhs=ones_mat,
                                 start=True, stop=True)
            xT_r = xT.rearrange("(a p) s -> p a s", p=P)
            # stage the DMAs for both chunks, then both stats blocks (sq on
            # DVE for chunk 0, on ACT for chunk 1 so neither engine queues
            # behind the other chunk's chain), then chain+normalize per chunk
            for c in range(S // NCK):
                sl = slice(c * NCK, (c + 1) * NCK)
                for j in range(_ND):
                    nc.sync.dma_start(out=xT_sb[:, j, sl], in_=xT_r[:, j, sl])
                if c == 0:
                    nc.sync.dma_start(out=cvec[:, 0:H], in_=binqk)
            nc.sync.dma_start(out=wqk_sb,
                              in_=wqkt.rearrange("(a p) e -> p a e", p=P))
            nc.sync.dma_start(out=wv_sb,
                              in_=wvt.rearrange("(a p) e -> p a e", p=P))
            nc.sync.dma_start(out=wout_sb,
                              in_=woutt.rearrange("(a p) e -> p a e", p=P))
            nc.sync.dma_start(out=resid_sb,
                              in_=resid.rearrange("(st p) e -> p st e", p=P))
            nc.gpsimd.dma_start(out=binv_bc,
                                in_=binv[None, :].to_broadcast((P, D)))
            sqs = {}

            def emit_sq(c, j):
                sl = slice(c * NCK, (c + 1) * NCK)
                sq = lntmp.tile([P, NCK], bf16, tag=f"sq{c}", bufs=8,
                                name=f"sq{c}_{j}")
                sqs[(c, j)] = sq
                with nc.allow_low_precision(reason="x^2 for LN stats"):
                    if c == 0:
                        nc.vector.tensor_tensor(out=sq, in0=xT_sb[:, j, sl],
                                                in1=xT_sb[:, j, sl], op=OP.mult)
                    else:
                        nc.scalar.square(out=sq, in_=xT_sb[:, j, sl])

            def emit_stats(c):
                sl = slice(c * NCK, (c + 1) * NCK)
                for j in range(_ND):
                    nc.tensor.matmul(sx_ps[:, sl], lhsT=ones_mat,
                                     rhs=xT_sb[:, j, sl],
                                     start=(j == 0), stop=(j == _ND - 1))
                    nc.tensor.matmul(sx2_ps[:, sl], lhsT=ones_mat,
                                     rhs=sqs[(c, j)],
                                     start=(j == 0), stop=(j == _ND - 1))

            chain_st = {}

            def emit_chain1(c):
                sl = slice(c * NCK, (c + 1) * NCK)
                with nc.allow_low_precision(reason="LN stats chain in bf16"):
                    mu_bc = lnrow.tile([P, NCK], bf16, tag="mu", bufs=2)
                    nc.vector.tensor_scalar_mul(mu_bc, sx_ps[:, sl], 1.0 / D)
                    var_bc = lnrow.tile([P, NCK], f32, tag="var", bufs=2)
                    nc.vector.tensor_scalar_mul(var_bc, sx2_ps[:, sl], 1.0 / D)
                    musq = lnrow.tile([P, NCK], bf16, tag="musq", bufs=2)
                    nc.vector.tensor_tensor(out=musq, in0=mu_bc, in1=mu_bc,
                                            op=OP.mult)
                    nc.vector.tensor_tensor(out=var_bc, in0=var_bc, in1=musq,
                                            op=OP.subtract)
                    chain_st[c] = (mu_bc, var_bc)

            def emit_sqrt(c):
                mu_bc, var_bc = chain_st[c]
                with nc.allow_low_precision(reason="LN std in bf16"):
                    std_bc = lnrow.tile([P, NCK], bf16, tag=f"std{c}", bufs=1)
                    nc.scalar.activation(out=std_bc, in_=var_bc, func=FT.Sqrt,
                                         bias=eps_sb)
                    chain_st[c] = (mu_bc, std_bc)

            def emit_xhat(c):
                sl = slice(c * NCK, (c + 1) * NCK)
                mu_bc, std_bc = chain_st[c]
                with nc.allow_low_precision(reason="LN normalize in bf16"):
                    b_bc = lnrow.tile([P, NCK], bf16, tag=f"b{c}", bufs=1)
                    nc.vector.reciprocal(out=b_bc, in_=std_bc)
                    mub_bc = lnrow.tile([P, NCK], bf16, tag=f"mub{c}", bufs=1)
                    nc.vector.tensor_tensor(out=mub_bc, in0=mu_bc, in1=b_bc,
                                            op=OP.mult)
                    for j in range(_ND):
                        t = lntmp.tile([P, NCK], bf16, tag="nrm", bufs=4)
                        eng = nc.vector if j < 5 else nc.gpsimd
                        eng.tensor_tensor(out=t, in0=xT_sb[:, j, sl],
                                          in1=b_bc, op=OP.mult)
                        eng.tensor_tensor(out=xn8[:, j, sl], in0=t,
                                          in1=mub_bc, op=OP.subtract)

            for j in range(_ND):
                emit_sq(0, j)
            for j in range(_ND // 2):
                emit_sq(1, j)
            emit_stats(0)
            emit_chain1(0)
            emit_sqrt(0)
            for j in range(_ND // 2, _ND):
                emit_sq(1, j)
            emit_xhat(0)
            emit_stats(1)
            emit_chain1(1)
            emit_sqrt(1)
            emit_xhat(1)

        # ============ Phases 2-4: projections + attention + out-proj ========
        with tc.tile_pool(name="expool", bufs=1) as expool, \
             tc.tile_pool(name="sidep", bufs=1) as sidep, \
             tc.tile_pool(name="mps", bufs=1, space="PSUM") as mps:

            def dr_matmul(ps_out, lhsT, rhs, start, stop):
                nc.tensor.matmul(ps_out, lhsT=lhsT, rhs=rhs, start=start,
                                 stop=stop, perf_mode=DR)

            def veng(i):
                return nc.vector if i % 2 == 0 else nc.gpsimd

            def emit_qk_half(et, half, on_act=False):
                # e-tile et (128 cols of q|k), s-half: one [128, 512] group
                ps = mps.tile([P, NCK], f32, tag="mm", bufs=2,
                              name=f"qk{et}_{half}")
                e0 = et * P
                for jp in range(_ND // 2):
                    for c2 in range(2):
                        sl = slice(half * NCK + c2 * 256,
                                   half * NCK + (c2 + 1) * 256)
                        dr_matmul(
                            ps[:, c2 * 256:(c2 + 1) * 256],
                            wqk_sb[:, 2 * jp:2 * jp + 2, e0:e0 + P],
                            xn8[:, 2 * jp:2 * jp + 2, sl],
                            start=(jp == 0 and c2 == 0),
                            stop=(jp == _ND // 2 - 1 and c2 == 1))
                sl = slice(half * NCK, (half + 1) * NCK)
                with nc.allow_low_precision(reason="qk to bf16"):
                    if on_act:
                        nc.scalar.activation(out=qkT[:, et, sl], in_=ps,
                                             func=FT.Identity,
                                             bias=binqk_sb[:, et:et + 1])
                    else:
                        nc.vector.tensor_scalar_add(qkT[:, et, sl], ps,
                                                    binqk_sb[:, et:et + 1])

            def emit_v_unit(st):
                # t-tile st: V natural [128 t, 512 e'] per e'-half
                for eh in range(2):
                    ps = mps.tile([P, NCK], f32, tag="mm", bufs=2,
                                  name=f"v{st}_{eh}")
                    t0 = st * P
                    for jp in range(_ND // 2):
                        for c2 in range(2):
                            sl = slice(eh * NCK + c2 * 256,
                                       eh * NCK + (c2 + 1) * 256)
                            dr_matmul(
                                ps[:, c2 * 256:(c2 + 1) * 256],
                                xn8[:, 2 * jp:2 * jp + 2, t0:t0 + P],
                                wv_sb[:, 2 * jp:2 * jp + 2, sl],
                                start=(jp == 0 and c2 == 0),
                                stop=(jp == _ND // 2 - 1 and c2 == 1))
                    with nc.allow_low_precision(reason="v to fp8"):
                        nc.vector.tensor_tensor(
                            out=v8[:, st, eh * 8:(eh + 1) * 8, :],
                            in0=ps.rearrange("p (h d) -> p h d", d=DH),
                            in1=binv_bc[:, eh * NCK:(eh + 1) * NCK]
                                .rearrange("p (h d) -> p h d", d=DH),
                            op=OP.add)

            def alloc_ex(hp):
                return expool.tile([P, 2, NS, S], fp8, tag="ex", bufs=2,
                                   name=f"ex{hp}")

            def emit_scores_piece(hp, ex_t, tts, shs):
                for tt in tts:
                    for idx in range(2):
                        base = idx * DH
                        ps = mps.tile([P, S], f32, tag="sc", bufs=2,
                                      name=f"sc{hp}_{tt}_{idx}_{shs[0]}")
                        for sh in shs:
                            sl = slice(sh * NCK, (sh + 1) * NCK)
                            nc.tensor.matmul(
                                ps[:, sl],
                                lhsT=qkT[base:base + DH, 8 + hp, tt * P:(tt + 1) * P],
                                rhs=qkT[base:base + DH, hp, sl],
                                start=True, stop=True, tile_position=(base, 0))
                        lo = shs[0] * NCK
                        hi = (shs[-1] + 1) * NCK
                        with nc.allow_low_precision(reason="exp to fp8"):
                            nc.scalar.activation(out=ex_t[:, idx, tt, lo:hi],
                                                 in_=ps[:, lo:hi], func=FT.Exp,
                                                 scale=EXP_SCALE, bias=expb)

            def emit_scores(hp):
                ex_t = alloc_ex(hp)
                emit_scores_piece(hp, ex_t, range(NS), (0, 1))
                return ex_t

            def emit_pvden(hp, ex_t):
                # PV + denominator (DoubleRow, planes = t-tile pairs), then
                # normalize ctx^T in fp8.  den rides the "mm" psum tag and
                # arrives replicated over 64 partitions (no broadcast needed).
                for sh in range(2):
                    for idx in range(2):
                        h = 2 * hp + idx
                        ctxps = mps.tile([DH, NCK], f32, tag="ctx", bufs=2,
                                         name=f"ctx{hp}_{sh}_{idx}")
                        denft = mps.tile([P, NCK], f32, tag="mm", bufs=2,
                                         name=f"den{hp}_{sh}_{idx}")
                        denps = denft[0:DH, :]
                        for ttp in range(NS // 2):
                            for c2 in range(2):
                                sl = slice(sh * NCK + c2 * 256,
                                           sh * NCK + (c2 + 1) * 256)
                                co = slice(c2 * 256, (c2 + 1) * 256)
                                st_ = (ttp == 0 and c2 == 0)
                                sp_ = (ttp == NS // 2 - 1 and c2 == 1)
                                dr_matmul(ctxps[:, co],
                                          v8[:, 2 * ttp:2 * ttp + 2, h, :],
                                          ex_t[:, idx, 2 * ttp:2 * ttp + 2, sl],
                                          start=st_, stop=sp_)
                                dr_matmul(denps[:, co], ones8_pl,
                                          ex_t[:, idx, 2 * ttp:2 * ttp + 2, sl],
                                          start=st_, stop=sp_)
                        sl = slice(sh * NCK, (sh + 1) * NCK)
                        rden = sidep.tile([DH, NCK], bf16, tag="rd", bufs=4,
                                          name=f"rd{hp}_{sh}_{idx}")
                        with nc.allow_low_precision(reason="denom in bf16"):
                            nc.vector.reciprocal(out=rden, in_=denps)
                            nc.vector.tensor_tensor(
                                out=ctxT8[idx * DH:(idx + 1) * DH, hp, sl],
                                in0=ctxps, in1=rden, op=OP.mult)

            def emit_outproj_a(sts):
                # heads 0-7 partial -> stage (residual folded in); runs in the
                # exp-bound era where PE and DVE have slack
                for st in sts:
                    s0 = st * P
                    for eh in range(2):
                        ps = mps.tile([P, NCK], f32, tag="mm", bufs=2,
                                      name=f"opa{st}_{eh}")
                        for hpp in range(2):
                            for c2 in range(2):
                                sl = slice(eh * NCK + c2 * 256,
                                           eh * NCK + (c2 + 1) * 256)
                                dr_matmul(
                                    ps[:, c2 * 256:(c2 + 1) * 256],
                                    ctxT8[:, 2 * hpp:2 * hpp + 2, s0:s0 + P],
                                    wout_sb[:, 2 * hpp:2 * hpp + 2, sl],
                                    start=(hpp == 0 and c2 == 0),
                                    stop=(hpp == 1 and c2 == 1))
                        sl = slice(eh * NCK, (eh + 1) * NCK)
                        with nc.allow_low_precision(reason="stage in bf16"):
                            nc.vector.scalar_tensor_tensor(
                                out=stage_sb[:, st, sl], in0=ps, scalar=OUT_SCALE,
                                in1=resid_sb[:, st, sl], op0=OP.mult, op1=OP.add)

            def emit_outproj_b():
                for st in range(NS):
                    ps = mps.tile([P, S], f32, tag="sc", bufs=2,
                                  name=f"opb{st}")
                    s0 = st * P
                    for eh in range(2):
                        for hpp in range(2, _ND // 2):
                            for c2 in range(2):
                                sl = slice(eh * NCK + c2 * 256,
                                           eh * NCK + (c2 + 1) * 256)
                                dr_matmul(
                                    ps[:, sl],
                                    ctxT8[:, 2 * hpp:2 * hpp + 2, s0:s0 + P],
                                    wout_sb[:, 2 * hpp:2 * hpp + 2, sl],
                                    start=(hpp == 2 and c2 == 0),
                                    stop=(hpp == _ND // 2 - 1 and c2 == 1))
                    ot = sidep.tile([P, S], bf16, tag="ot", bufs=2,
                                    name=f"ot{st}")
                    ob = sidep.tile([P, S], bf16, tag="ob", bufs=2,
                                    name=f"ob{st}")
                    with nc.allow_low_precision(reason="out in bf16"):
                        nc.scalar.mul(ot, ps, OUT_SCALE)
                        nc.vector.tensor_tensor(out=ob, in0=ot,
                                                in1=stage_sb[:, st, :],
                                                op=OP.add)
                    nc.sync.dma_start(out=out[st * P:(st + 1) * P, :], in_=ob)

            # ---- interleaved emission ----
            # s-half-0 work first: runs while LayerNorm's second chunk is
            # still on DVE, keeping the PE busy
            # pair-0 q/k s-half-0 first: scores for t-tiles 0-3 x s-half-0
            # depend only on LN chunk 0, so exp starts ~20us earlier
            emit_qk_half(0, 0, on_act=True)
            emit_qk_half(8, 0, on_act=True)
            ex_prev = alloc_ex(0)
            emit_scores_piece(0, ex_prev, range(NS // 2), (0,))
            emit_qk_half(0, 1, on_act=True)
            emit_qk_half(8, 1, on_act=True)
            emit_scores_piece(0, ex_prev, range(NS // 2), (1,))
            emit_scores_piece(0, ex_prev, range(NS // 2, NS), (0, 1))
            for half in range(2):
                emit_qk_half(1, half)
                emit_qk_half(9, half)
            ex_cur = emit_scores(1)
            for st in range(NS):
                emit_v_unit(st)
            emit_pvden(0, ex_prev)
            ex_prev = ex_cur
            for p in range(2, H // 2):
                for half in range(2):
                    emit_qk_half(p, half)
                    emit_qk_half(8 + p, half)
                ex_cur = emit_scores(p)
                emit_pvden(p - 1, ex_prev)
                ex_prev = ex_cur
                if p >= 4:
                    emit_outproj_a(range(2 * (p - 4), 2 * (p - 3)))
            emit_pvden(H // 2 - 1, ex_prev)
            emit_outproj_b()


def build_nc():
    import concourse.bacc as bacc
    import concourse.tile as tile
    from concourse import mybir

    f32 = mybir.dt.float32
    bf16 = mybir.dt.bfloat16
    fp8 = mybir.dt.float8e4

    nc = bacc.Bacc("TRN2", target_bir_lowering=False, debug=False)
    aps = {
        "xt": nc.dram_tensor("xt", [D, S], bf16, kind="ExternalInput").ap(),
        "resid": nc.dram_tensor("resid", [S, D], bf16, kind="ExternalInput").ap(),
        "wqkt": nc.dram_tensor("wqkt", [D, 2 * D], fp8, kind="ExternalInput").ap(),
        "wvt": nc.dram_tensor("wvt", [D, D], fp8, kind="ExternalInput").ap(),
        "woutt": nc.dram_tensor("woutt", [D, D], fp8, kind="ExternalInput").ap(),
        "binqk": nc.dram_tensor("binqk", [P, H], f32, kind="ExternalInput").ap(),
        "binv": nc.dram_tensor("binv", [D], f32, kind="ExternalInput").ap(),
        "out": nc.dram_tensor("out", [S, D], bf16, kind="ExternalOutput").ap(),
    }
    with tile.TileContext(nc) as tc:
        _emit(tc, aps)
    nc.compile()
    return nc


def prep_inputs(x, ln_gamma, ln_beta, in_proj_w, in_proj_b, out_proj_w, out_proj_b,
                n_cores=N_CORES):
    bf = ml_dtypes.bfloat16
    f8 = ml_dtypes.float8_e4m3
    f32c = lambda a: np.ascontiguousarray(a, dtype=np.float32)
    win = np.asarray(in_proj_w, np.float32)
    g = np.asarray(ln_gamma, np.float32)
    bt = np.asarray(ln_beta, np.float32)
    bin_ = np.asarray(in_proj_b, np.float32)
    wing = win * g[None, :]          # gamma folded into in-proj columns
    binf = bin_ + win @ bt           # beta folded into the in-proj biases
    shared = {
        "wqkt": np.ascontiguousarray((wing[:2 * D] * WS).T).astype(f8),
        "wvt": np.ascontiguousarray((wing[2 * D:] * WS).T).astype(f8),
        "woutt": np.ascontiguousarray(np.asarray(out_proj_w, np.float32).T * WS).astype(f8),
        "binqk": f32c((binf[:2 * D] * WS).reshape(H, P).T),
        "binv": f32c(binf[2 * D:] * WS),
    }
    bout = np.asarray(out_proj_b, np.float32)
    in_maps = []
    for i in range(n_cores):
        xi = np.asarray(x[i], np.float32)
        m = dict(shared)
        m["xt"] = np.ascontiguousarray(xi.T).astype(bf)
        m["resid"] = np.ascontiguousarray(xi + bout).astype(bf)
        in_maps.append(m)
    return in_maps


def kernel(x, ln_gamma, ln_beta, in_proj_w, in_proj_b, out_proj_w, out_proj_b):
    global LAST_RESULTS
    from concourse import bass_utils

    if "nc" not in _NC_CACHE:
        _NC_CACHE["nc"] = build_nc()
    nc = _NC_CACHE["nc"]

    in_maps = prep_inputs(x, ln_gamma, ln_beta, in_proj_w, in_proj_b,
                          out_proj_w, out_proj_b)
    res = bass_utils.run_bass_kernel_spmd(nc, in_maps, core_ids=list(range(N_CORES)))
    LAST_RESULTS = res
    out = np.stack([r["out"] for r in res.results], axis=0)
    return np.ascontiguousarray(out, dtype=np.float32)


# revision 20
# speedup vs baseline: 1.0754x; 1.0321x over previous
"""Self-contained Trainium2 Bass kernel: pre-LN multi-head attention block.

Computes, for x [B=8, S=1024, D=1024] (fp32) and packed attention weights:
    out = x + out_proj(MHA(LayerNorm(x)))
matching torch nn.MultiheadAttention's explicit (non-flash) path with 16 heads.

Sharding: data-parallel over batch - core i handles batch element i; no
collectives, outputs are concatenated on the host.

Per-core strategy (fp8 DoubleRow matmuls at 2x PE throughput):
  - LN runs on transposed activations (d on partitions); stats are matmuls
    against an all-ones stationary so the sums land partition-replicated in
    PSUM; the normalize chain runs in bf16 on DVE and the gamma/beta apply
    runs on the Pool engine (tensor_scalar), writing xn directly in fp8.
  - QKV / V / PV / out-proj all run as fp8e4 DoubleRow matmuls with full
    128-wide stationaries: weights are pre-scaled by 32 on the host (power
    of two; folded back out via the softmax exp scale and the final output
    scale); each instruction contracts 2x128 d-coords at 0.5 cycles per
    output column.
  - scores^T[t,s] = K^T.T @ Q^T per head stay bf16 (K=64 contraction gains
    nothing from DoubleRow); exp runs on the scalar engine over [128, 1024]
    PSUM tiles (amortizing the fixed ACT access latency) with scale 1/8192
    and a -3 offset (cancels in softmax; keeps fp8 exp in range).
  - the softmax denominator comes from a DoubleRow matmul against an fp8
    all-ones stationary - its [64, N] output is the denominator replicated
    across 64 partitions, so the per-head normalize needs no broadcast.
  - PE emission: Q/K/V units are split by s-half so the first halves (plus
    warmup matmuls) keep the PE busy while LayerNorm finishes the second
    x chunk; per head pair the stream is qk(p+1) | scores(p) | pv+den(p-1)
    so the scalar engine's exp stream overlaps PE work throughout.
  - PSUM->SBUF copies alternate between DVE and the otherwise-idle Pool
    engine; residual + out_proj bias are pre-added on the host (bf16) and
    merged with one fused scalar_tensor_tensor: (psum * 2^-10) + resid.
"""

import numpy as np
import ml_dtypes

P = 128
D = 1024
H = 16
DH = 64
B = 8
S = 1024
LN_EPS = 1e-5
N_CORES = 8

_ND = D // P   # d tiles (8)
NS = S // P    # s tiles (8)
NCK = 512      # LN chunk / matmul moving width
WS = 32.0      # fp8 weight pre-scale (power of two)
EXP_SCALE = 0.125 / (WS * WS)   # 1/8192: folds 1/sqrt(dh) and the q/k scales
EXP_BIAS = -3.0                 # cancels in softmax; keeps fp8 exp in range
OUT_SCALE = 1.0 / (WS * WS)     # folds the v/out-proj weight scales back out

LAST_RESULTS = None
_NC_CACHE = {}


def _emit(tc, aps):
    from concourse import mybir

    nc = tc.nc
    f32 = mybir.dt.float32
    bf16 = mybir.dt.bfloat16
    fp8 = mybir.dt.float8e4
    FT = mybir.ActivationFunctionType
    OP = mybir.AluOpType
    DR = mybir.MatmulPerfMode.DoubleRow

    xT, resid, wqkt, wvt, woutt, binqk, negw1, binv, out = (
        aps["xt"], aps["resid"], aps["wqkt"], aps["wvt"], aps["woutt"],
        aps["binqk"], aps["negw1"], aps["binv"], aps["out"],
    )

    with tc.tile_pool(name="consts", bufs=1) as consts, \
         tc.tile_pool(name="acts", bufs=1) as acts, \
         tc.tile_pool(name="wpool", bufs=1) as wpool:

        # ---------- constants (DMAs issued after the first x chunk) ----------
        cvec = consts.tile([P, 2 * H + 1], f32)
        binqk_sb = cvec[:, 0:H]
        negw1_sb = cvec[:, H:2 * H]
        eps_sb = cvec[:, 2 * H:2 * H + 1]
        nc.vector.memset(eps_sb, LN_EPS)
        ones_mat = consts.tile([P, P], bf16)
        nc.vector.memset(ones_mat, 1.0)
        ones8 = consts.tile([P, P], fp8)
        nc.vector.memset(ones8, 1.0)
        ones8_pl = ones8.rearrange("p (a m) -> p a m", a=2)  # [P, 2, 64]
        expb = consts.tile([P, 1], f32)
        nc.vector.memset(expb, EXP_BIAS)
        binv_bc = consts.tile([P, D], f32)

        # ---------- persistent activations ----------
        # staged out-proj partial (heads 0-7), with residual folded in
        xn8 = acts.tile([P, _ND, S], fp8)        # normalized x, transposed
        qkT = acts.tile([P, 2 * _ND, S], bf16)   # q tiles 0..7, k tiles 8..15
        v8 = acts.tile([P, NS, H, DH], fp8)      # v natural [t, h, dh]
        ctxT8 = acts.tile([P, _ND, S], fp8)      # normalized ctx^T (d on part)
        resid_sb = acts.tile([P, NS, D], bf16)   # x + out_proj_b, natural
        stage_sb = acts.tile([P, NS, D], bf16)
        xT_sb = acts.tile([P, _ND, S], bf16)     # raw x^T (bf16) for LN + the
                                                 # direct q/k path of pairs 0-1
        b_sb = acts.tile([P, S], bf16)           # rstd, partition-replicated
        mub_sb = acts.tile([P, S], bf16)         # mu*rstd, partition-replicated

        # ---------- weights (fp8, pre-scaled by WS on host) ----------
        wqk_sb = wpool.tile([P, _ND, 2 * D], fp8)
        wv_sb = wpool.tile([P, _ND, D], fp8)
        wout_sb = wpool.tile([P, _ND, D], fp8)

        # ================= Phase 1: LayerNorm =================
        with tc.tile_pool(name="lnsb", bufs=1) as lnsb, \
             tc.tile_pool(name="lnrow", bufs=1) as lnrow, \
             tc.tile_pool(name="lntmp", bufs=2) as lntmp, \
             tc.tile_pool(name="lnps", bufs=1, space="PSUM") as lnps:
            sx_ps = lnps.tile([P, S], f32, tag="sx")
            sx2_ps = lnps.tile([P, S], f32, tag="sx2")
            # PE p-state warmup: dummy matmuls chain into the LN stats so the
            # clock is at full speed (and stays there) when real work arrives
            warm_ps = lnps.tile([P, P], f32, tag="warm")
            for _ in range(24):
                nc.tensor.matmul(warm_ps, lhsT=ones_mat, rhs=ones_mat,
                                 start=True, stop=True)
            xT_r = xT.rearrange("(a p) s -> p a s", p=P)
            # stage the DMAs for both chunks, then both stats blocks (sq on
            # DVE for chunk 0, on ACT for chunk 1 so neither engine queues
            # behind the other chunk's chain), then chain+normalize per chunk
            for c in range(S // NCK):
                sl = slice(c * NCK, (c + 1) * NCK)
                for j in range(_ND):
                    nc.sync.dma_start(out=xT_sb[:, j, sl], in_=xT_r[:, j, sl])
                if c == 0:
                    nc.sync.dma_start(out=cvec[:, 0:H], in_=binqk)
                    nc.sync.dma_start(out=cvec[:, H:2 * H], in_=negw1)
                    # pair-0/1 q,k weight slices first: the direct-from-x
                    # path needs them ~10us before the bulk
                    wqkt_r = wqkt.rearrange("(a p) e -> p a e", p=P)
                    for et in (0, 8, 1, 9):
                        nc.sync.dma_start(out=wqk_sb[:, :, et * P:(et + 1) * P],
                                          in_=wqkt_r[:, :, et * P:(et + 1) * P])
            wqkt_r2 = wqkt.rearrange("(a p) e -> p a e", p=P)
            nc.sync.dma_start(out=wqk_sb[:, :, 2 * P:8 * P],
                              in_=wqkt_r2[:, :, 2 * P:8 * P])
            nc.sync.dma_start(out=wqk_sb[:, :, 10 * P:16 * P],
                              in_=wqkt_r2[:, :, 10 * P:16 * P])
            nc.sync.dma_start(out=wv_sb,
                              in_=wvt.rearrange("(a p) e -> p a e", p=P))
            nc.sync.dma_start(out=wout_sb,
                              in_=woutt.rearrange("(a p) e -> p a e", p=P))
            nc.sync.dma_start(out=resid_sb,
                              in_=resid.rearrange("(st p) e -> p st e", p=P))
            nc.gpsimd.dma_start(out=binv_bc,
                                in_=binv[None, :].to_broadcast((P, D)))
            sqs = {}

            def emit_sq(c, j):
                sl = slice(c * NCK, (c + 1) * NCK)
                sq = lntmp.tile([P, NCK], bf16, tag=f"sq{c}", bufs=8,
                                name=f"sq{c}_{j}")
                sqs[(c, j)] = sq
                with nc.allow_low_precision(reason="x^2 for LN stats"):
                    if c == 0:
                        nc.vector.tensor_tensor(out=sq, in0=xT_sb[:, j, sl],
                                                in1=xT_sb[:, j, sl], op=OP.mult)
                    else:
                        nc.scalar.square(out=sq, in_=xT_sb[:, j, sl])

            def emit_stats(c):
                sl = slice(c * NCK, (c + 1) * NCK)
                for j in range(_ND):
                    nc.tensor.matmul(sx_ps[:, sl], lhsT=ones_mat,
                                     rhs=xT_sb[:, j, sl],
                                     start=(j == 0), stop=(j == _ND - 1))
                    nc.tensor.matmul(sx2_ps[:, sl], lhsT=ones_mat,
                                     rhs=sqs[(c, j)],
                                     start=(j == 0), stop=(j == _ND - 1))

            chain_st = {}

            def emit_chain1(c):
                sl = slice(c * NCK, (c + 1) * NCK)
                with nc.allow_low_precision(reason="LN stats chain in bf16"):
                    mu_bc = lnrow.tile([P, NCK], bf16, tag="mu", bufs=2)
                    nc.vector.tensor_scalar_mul(mu_bc, sx_ps[:, sl], 1.0 / D)
                    var_bc = lnrow.tile([P, NCK], f32, tag="var", bufs=2)
                    nc.vector.tensor_scalar_mul(var_bc, sx2_ps[:, sl], 1.0 / D)
                    musq = lnrow.tile([P, NCK], bf16, tag="musq", bufs=2)
                    nc.vector.tensor_tensor(out=musq, in0=mu_bc, in1=mu_bc,
                                            op=OP.mult)
                    nc.vector.tensor_tensor(out=var_bc, in0=var_bc, in1=musq,
                                            op=OP.subtract)
                    chain_st[c] = (mu_bc, var_bc)

            def emit_sqrt(c):
                mu_bc, var_bc = chain_st[c]
                with nc.allow_low_precision(reason="LN std in bf16"):
                    std_bc = lnrow.tile([P, NCK], bf16, tag=f"std{c}", bufs=1)
                    nc.scalar.activation(out=std_bc, in_=var_bc, func=FT.Sqrt,
                                         bias=eps_sb)
                    chain_st[c] = (mu_bc, std_bc)

            def emit_rstd(c):
                sl = slice(c * NCK, (c + 1) * NCK)
                mu_bc, std_bc = chain_st[c]
                with nc.allow_low_precision(reason="LN normalize in bf16"):
                    nc.vector.reciprocal(out=b_sb[:, sl], in_=std_bc)
                    nc.vector.tensor_tensor(out=mub_sb[:, sl], in0=mu_bc,
                                            in1=b_sb[:, sl], op=OP.mult)

            for j in range(_ND):
                emit_sq(0, j)
            for j in range(_ND // 2):
                emit_sq(1, j)
            emit_stats(0)
            emit_chain1(0)
            emit_sqrt(0)
            for j in range(_ND // 2, _ND):
                emit_sq(1, j)
            emit_rstd(0)
            emit_stats(1)
            emit_chain1(1)
            emit_sqrt(1)
            emit_rstd(1)

        # ============ Phases 2-4: projections + attention + out-proj ========
        with tc.tile_pool(name="expool", bufs=1) as expool, \
             tc.tile_pool(name="sidep", bufs=1) as sidep, \
             tc.tile_pool(name="mps", bufs=1, space="PSUM") as mps:

            def dr_matmul(ps_out, lhsT, rhs, start, stop):
                nc.tensor.matmul(ps_out, lhsT=lhsT, rhs=rhs, start=start,
                                 stop=stop, perf_mode=DR)

            def veng(i):
                return nc.vector if i % 2 == 0 else nc.gpsimd

            def emit_xhat(c):
                # xn8 = (x - mu) * rstd in fp8; feeds the DoubleRow paths
                # (qk pairs 2-7 and V); off the critical path by now
                sl = slice(c * NCK, (c + 1) * NCK)
                with nc.allow_low_precision(reason="LN normalize in bf16"):
                    for j in range(_ND):
                        t = sidep.tile([P, NCK], bf16, tag="nrm", bufs=4,
                                       name=f"nrm{c}_{j}")
                        eng = nc.vector if j < 4 else nc.gpsimd
                        eng.tensor_tensor(out=t, in0=xT_sb[:, j, sl],
                                          in1=b_sb[:, sl], op=OP.mult)
                        eng.tensor_tensor(out=xn8[:, j, sl], in0=t,
                                          in1=mub_sb[:, sl], op=OP.subtract)

            def emit_qk_direct(et, half):
                # q/k from raw bf16 x^T with the LN folded in afterwards:
                # qkT = b * (x @ W') - (mu*b) * w1   (w1 = col-sums of W')
                ps = mps.tile([P, NCK], f32, tag="mm", bufs=2,
                              name=f"qd{et}_{half}")
                sl = slice(half * NCK, (half + 1) * NCK)
                for j in range(_ND):
                    nc.tensor.matmul(ps, lhsT=wqk_sb[:, j, et * P:(et + 1) * P],
                                     rhs=xT_sb[:, j, sl],
                                     start=(j == 0), stop=(j == _ND - 1))
                with nc.allow_low_precision(reason="qk to bf16"):
                    t = sidep.tile([P, NCK], bf16, tag="qd", bufs=4,
                                   name=f"qdt{et}_{half}")
                    nc.vector.tensor_tensor(out=t, in0=ps, in1=b_sb[:, sl],
                                            op=OP.mult)
                    nc.vector.scalar_tensor_tensor(
                        out=qkT[:, et, sl], in0=mub_sb[:, sl],
                        scalar=negw1_sb[:, et:et + 1], in1=t,
                        op0=OP.mult, op1=OP.add)

            def emit_qk_half(et, half, on_act=False):
                # e-tile et (128 cols of q|k), s-half: one [128, 512] group
                ps = mps.tile([P, NCK], f32, tag="mm", bufs=2,
                              name=f"qk{et}_{half}")
                e0 = et * P
                for jp in range(_ND // 2):
                    for c2 in range(2):
                        sl = slice(half * NCK + c2 * 256,
                                   half * NCK + (c2 + 1) * 256)
                        dr_matmul(
                            ps[:, c2 * 256:(c2 + 1) * 256],
                            wqk_sb[:, 2 * jp:2 * jp + 2, e0:e0 + P],
                            xn8[:, 2 * jp:2 * jp + 2, sl],
                            start=(jp == 0 and c2 == 0),
                            stop=(jp == _ND // 2 - 1 and c2 == 1))
                sl = slice(half * NCK, (half + 1) * NCK)
                with nc.allow_low_precision(reason="qk to bf16"):
                    if on_act:
                        nc.scalar.activation(out=qkT[:, et, sl], in_=ps,
                                             func=FT.Identity,
                                             bias=binqk_sb[:, et:et + 1])
                    else:
                        nc.vector.tensor_scalar_add(qkT[:, et, sl], ps,
                                                    binqk_sb[:, et:et + 1])

            def emit_v_unit(st):
                # t-tile st: V natural [128 t, 512 e'] per e'-half
                for eh in range(2):
                    ps = mps.tile([P, NCK], f32, tag="mm", bufs=2,
                                  name=f"v{st}_{eh}")
                    t0 = st * P
                    for jp in range(_ND // 2):
                        for c2 in range(2):
                            sl = slice(eh * NCK + c2 * 256,
                                       eh * NCK + (c2 + 1) * 256)
                            dr_matmul(
                                ps[:, c2 * 256:(c2 + 1) * 256],
                                xn8[:, 2 * jp:2 * jp + 2, t0:t0 + P],
                                wv_sb[:, 2 * jp:2 * jp + 2, sl],
                                start=(jp == 0 and c2 == 0),
                                stop=(jp == _ND // 2 - 1 and c2 == 1))
                    with nc.allow_low_precision(reason="v to fp8"):
                        nc.vector.tensor_tensor(
                            out=v8[:, st, eh * 8:(eh + 1) * 8, :],
                            in0=ps.rearrange("p (h d) -> p h d", d=DH),
                            in1=binv_bc[:, eh * NCK:(eh + 1) * NCK]
                                .rearrange("p (h d) -> p h d", d=DH),
                            op=OP.add)

            def alloc_ex(hp):
                return expool.tile([P, 2, NS, S], fp8, tag="ex", bufs=2,
                                   name=f"ex{hp}")

            def emit_scores_piece(hp, ex_t, tts, shs):
                for tt in tts:
                    for idx in range(2):
                        base = idx * DH
                        ps = mps.tile([P, S], f32, tag="sc", bufs=2,
                                      name=f"sc{hp}_{tt}_{idx}_{shs[0]}")
                        for sh in shs:
                            sl = slice(sh * NCK, (sh + 1) * NCK)
                            nc.tensor.matmul(
                                ps[:, sl],
                                lhsT=qkT[base:base + DH, 8 + hp, tt * P:(tt + 1) * P],
                                rhs=qkT[base:base + DH, hp, sl],
                                start=True, stop=True, tile_position=(base, 0))
                        lo = shs[0] * NCK
                        hi = (shs[-1] + 1) * NCK
                        with nc.allow_low_precision(reason="exp to fp8"):
                            nc.scalar.activation(out=ex_t[:, idx, tt, lo:hi],
                                                 in_=ps[:, lo:hi], func=FT.Exp,
                                                 scale=EXP_SCALE, bias=expb)

            def emit_scores(hp):
                ex_t = alloc_ex(hp)
                emit_scores_piece(hp, ex_t, range(NS), (0, 1))
                return ex_t

            def emit_pvden(hp, ex_t):
                # PV + denominator (DoubleRow, planes = t-tile pairs), then
                # normalize ctx^T in fp8.  den rides the "mm" psum tag and
                # arrives replicated over 64 partitions (no broadcast needed).
                for sh in range(2):
                    for idx in range(2):
                        h = 2 * hp + idx
                        ctxps = mps.tile([DH, NCK], f32, tag="ctx", bufs=2,
                                         name=f"ctx{hp}_{sh}_{idx}")
                        denft = mps.tile([P, NCK], f32, tag="mm", bufs=2,
                                         name=f"den{hp}_{sh}_{idx}")
                        denps = denft[0:DH, :]
                        for ttp in range(NS // 2):
                            for c2 in range(2):
                                sl = slice(sh * NCK + c2 * 256,
                                           sh * NCK + (c2 + 1) * 256)
                                co = slice(c2 * 256, (c2 + 1) * 256)
                                st_ = (ttp == 0 and c2 == 0)
                                sp_ = (ttp == NS // 2 - 1 and c2 == 1)
                                dr_matmul(ctxps[:, co],
                                          v8[:, 2 * ttp:2 * ttp + 2, h, :],
                                          ex_t[:, idx, 2 * ttp:2 * ttp + 2, sl],
                                          start=st_, stop=sp_)
                                dr_matmul(denps[:, co], ones8_pl,
                                          ex_t[:, idx, 2 * ttp:2 * ttp + 2, sl],
                                          start=st_, stop=sp_)
                        sl = slice(sh * NCK, (sh + 1) * NCK)
                        rden = sidep.tile([DH, NCK], bf16, tag="rd", bufs=4,
                                          name=f"rd{hp}_{sh}_{idx}")
                        with nc.allow_low_precision(reason="denom in bf16"):
                            nc.vector.reciprocal(out=rden, in_=denps)
                            nc.vector.tensor_tensor(
                                out=ctxT8[idx * DH:(idx + 1) * DH, hp, sl],
                                in0=ctxps, in1=rden, op=OP.mult)

            def emit_outproj_a(sts):
                # heads 0-7 partial -> stage (residual folded in); runs in the
                # exp-bound era where PE and DVE have slack
                for st in sts:
                    s0 = st * P
                    for eh in range(2):
                        ps = mps.tile([P, NCK], f32, tag="mm", bufs=2,
                                      name=f"opa{st}_{eh}")
                        for hpp in range(2):
                            for c2 in range(2):
                                sl = slice(eh * NCK + c2 * 256,
                                           eh * NCK + (c2 + 1) * 256)
                                dr_matmul(
                                    ps[:, c2 * 256:(c2 + 1) * 256],
                                    ctxT8[:, 2 * hpp:2 * hpp + 2, s0:s0 + P],
                                    wout_sb[:, 2 * hpp:2 * hpp + 2, sl],
                                    start=(hpp == 0 and c2 == 0),
                                    stop=(hpp == 1 and c2 == 1))
                        sl = slice(eh * NCK, (eh + 1) * NCK)
                        with nc.allow_low_precision(reason="stage in bf16"):
                            nc.vector.scalar_tensor_tensor(
                                out=stage_sb[:, st, sl], in0=ps, scalar=OUT_SCALE,
                                in1=resid_sb[:, st, sl], op0=OP.mult, op1=OP.add)

            def emit_outproj_b():
                for st in range(NS):
                    ps = mps.tile([P, S], f32, tag="sc", bufs=2,
                                  name=f"opb{st}")
                    s0 = st * P
                    for eh in range(2):
                        for hpp in range(2, _ND // 2):
                            for c2 in range(2):
                                sl = slice(eh * NCK + c2 * 256,
                                           eh * NCK + (c2 + 1) * 256)
                                dr_matmul(
                                    ps[:, sl],
                                    ctxT8[:, 2 * hpp:2 * hpp + 2, s0:s0 + P],
                                    wout_sb[:, 2 * hpp:2 * hpp + 2, sl],
                                    start=(hpp == 2 and c2 == 0),
                                    stop=(hpp == _ND // 2 - 1 and c2 == 1))
                    ot = sidep.tile([P, S], bf16, tag="ot", bufs=2,
                                    name=f"ot{st}")
                    ob = sidep.tile([P, S], bf16, tag="ob", bufs=2,
                                    name=f"ob{st}")
                    with nc.allow_low_precision(reason="out in bf16"):
                        nc.scalar.mul(ot, ps, OUT_SCALE)
                        nc.vector.tensor_tensor(out=ob, in0=ot,
                                                in1=stage_sb[:, st, :],
                                                op=OP.add)
                    nc.sync.dma_start(out=out[st * P:(st + 1) * P, :], in_=ob)

            # ---- interleaved emission ----
            # s-half-0 work first: runs while LayerNorm's second chunk is
            # still on DVE, keeping the PE busy
            # pairs 0/1 q/k straight from x (no xhat dependency): scores for
            # t-tiles 0-3 x s-half-0 only need LN chunk 0's stats
            emit_qk_direct(0, 0)
            emit_qk_direct(8, 0)
            ex_prev = alloc_ex(0)
            emit_scores_piece(0, ex_prev, range(NS // 2), (0,))
            emit_qk_direct(0, 1)
            emit_qk_direct(8, 1)
            emit_scores_piece(0, ex_prev, range(NS // 2), (1,))
            emit_scores_piece(0, ex_prev, range(NS // 2, NS), (0, 1))
            for half in range(2):
                emit_qk_direct(1, half)
                emit_qk_direct(9, half)
            ex_cur = emit_scores(1)
            emit_xhat(0)
            emit_xhat(1)
            for st in range(NS):
                emit_v_unit(st)
            emit_pvden(0, ex_prev)
            ex_prev = ex_cur
            for p in range(2, H // 2):
                for half in range(2):
                    emit_qk_half(p, half)
                    emit_qk_half(8 + p, half)
                ex_cur = emit_scores(p)
                emit_pvden(p - 1, ex_prev)
                ex_prev = ex_cur
                if p >= 4:
                    emit_outproj_a(range(2 * (p - 4), 2 * (p - 3)))
            emit_pvden(H // 2 - 1, ex_prev)
            emit_outproj_b()


def build_nc():
    import concourse.bacc as bacc
    import concourse.tile as tile
    from concourse import mybir

    f32 = mybir.dt.float32
    bf16 = mybir.dt.bfloat16
    fp8 = mybir.dt.float8e4

    nc = bacc.Bacc("TRN2", target_bir_lowering=False, debug=False)
    aps = {
        "xt": nc.dram_tensor("xt", [D, S], bf16, kind="ExternalInput").ap(),
        "resid": nc.dram_tensor("resid", [S, D], bf16, kind="ExternalInput").ap(),
        "wqkt": nc.dram_tensor("wqkt", [D, 2 * D], fp8, kind="ExternalInput").ap(),
        "wvt": nc.dram_tensor("wvt", [D, D], fp8, kind="ExternalInput").ap(),
        "woutt": nc.dram_tensor("woutt", [D, D], fp8, kind="ExternalInput").ap(),
        "binqk": nc.dram_tensor("binqk", [P, H], f32, kind="ExternalInput").ap(),
        "negw1": nc.dram_tensor("negw1", [P, H], f32, kind="ExternalInput").ap(),
        "binv": nc.dram_tensor("binv", [D], f32, kind="ExternalInput").ap(),
        "out": nc.dram_tensor("out", [S, D], bf16, kind="ExternalOutput").ap(),
    }
    with tile.TileContext(nc) as tc:
        _emit(tc, aps)
    nc.compile()
    return nc


def prep_inputs(x, ln_gamma, ln_beta, in_proj_w, in_proj_b, out_proj_w, out_proj_b,
                n_cores=N_CORES):
    bf = ml_dtypes.bfloat16
    f8 = ml_dtypes.float8_e4m3
    f32c = lambda a: np.ascontiguousarray(a, dtype=np.float32)
    win = np.asarray(in_proj_w, np.float32)
    g = np.asarray(ln_gamma, np.float32)
    bt = np.asarray(ln_beta, np.float32)
    bin_ = np.asarray(in_proj_b, np.float32)
    wing = win * g[None, :]          # gamma folded into in-proj columns
    binf = bin_ + win @ bt           # beta folded into the in-proj biases
    wqkt8 = np.ascontiguousarray((wing[:2 * D] * WS).T).astype(f8)
    negw1 = -wqkt8.astype(np.float32).sum(axis=0)  # [2D]
    shared = {
        "wqkt": wqkt8,
        "negw1": f32c(negw1.reshape(H, P).T),
        "wvt": np.ascontiguousarray((wing[2 * D:] * WS).T).astype(f8),
        "woutt": np.ascontiguousarray(np.asarray(out_proj_w, np.float32).T * WS).astype(f8),
        "binqk": f32c((binf[:2 * D] * WS).reshape(H, P).T),
        "binv": f32c(binf[2 * D:] * WS),
    }
    bout = np.asarray(out_proj_b, np.float32)
    in_maps = []
    for i in range(n_cores):
        xi = np.asarray(x[i], np.float32)
        m = dict(shared)
        m["xt"] = np.ascontiguousarray(xi.T).astype(bf)
        m["resid"] = np.ascontiguousarray(xi + bout).astype(bf)
        in_maps.append(m)
    return in_maps


def kernel(x, ln_gamma, ln_beta, in_proj_w, in_proj_b, out_proj_w, out_proj_b):
    global LAST_RESULTS
    from concourse import bass_utils

    if "nc" not in _NC_CACHE:
        _NC_CACHE["nc"] = build_nc()
    nc = _NC_CACHE["nc"]

    in_maps = prep_inputs(x, ln_gamma, ln_beta, in_proj_w, in_proj_b,
                          out_proj_w, out_proj_b)
    res = bass_utils.run_bass_kernel_spmd(nc, in_maps, core_ids=list(range(N_CORES)))
    LAST_RESULTS = res
    out = np.stack([r["out"] for r in res.results], axis=0)
    return np.ascontiguousarray(out, dtype=np.float32)
